# revision 51
# baseline (speedup 1.0000x reference)
"""Causal multi-head self-attention (S=4096, D=1024, H=16, RoPE) on 8 trn2 cores.

Tensor-parallel over heads: core c owns heads 2c, 2c+1.
Pipeline per core:
  A) xT shipped pre-transposed from host; per-tile block loads
  B) q/k projections bf16 -> RoPE on DVE -> q stored fp8 (duplicated),
     k stored dual-fp8 (hi + lo residual) for DoubleRow score matmuls
  C) flash-style causal attention: scores via fp8 DoubleRow matmul
     (k_hi.T q + k_lo.T q, half cost), exp on ACT, PV in bf16;
     denominator via ones-column in v; normalize -> headsT
  D) 4 range-wise AllGathers of headsT; ALL output projections deferred
     to overlap the final collective's 15us constant cost
Host assembles out[0, :, 128c:128c+128] = outT_c.T
"""

import sys

for _p in ("/opt/trn_rl_repo", "/root/.axon_site/_ro/trn_rl_repo"):
    if _p not in sys.path:
        sys.path.append(_p)

import numpy as np
import ml_dtypes

import concourse.bass as bass
import concourse.tile as tile
from concourse import bacc, mybir
from concourse.bass_utils import run_bass_kernel_spmd

BF16 = mybir.dt.bfloat16
F32 = mybir.dt.float32
F8 = mybir.dt.float8e4
NPBF16 = ml_dtypes.bfloat16
NPF8 = ml_dtypes.float8_e4m3
DR = mybir.MatmulPerfMode.DoubleRow

S = 4096          # sequence
D = 1024          # model dim
NH = 16           # heads
DK = 64           # head dim
NCORES = 8
HPC = NH // NCORES          # 2 heads per core
P = HPC * DK                # 128 = per-core head feature count
THETA = 10000.0
ST = 512                    # s-tile width (a-tile width too)
NT = S // ST                # 8 tiles
EXPFN = mybir.ActivationFunctionType.Exp
WSCALE = 64.0               # fp8 weight pre-scale (keeps w_lo out of subnormals)
ATT_SCALE = 1.0 / 8.0       # 1/sqrt(DK), folded into the exp activation

# heads-allgather ranges: (first s-tile, n s-tiles). Output projections all
# run after the final gather is issued, filling its 15us constant cost.
RANGES = [(0, 4), (4, 2), (6, 1), (7, 1)]
GATHER_AT = {3: 0, 5: 1, 6: 2, 7: 3}   # t -> range k gathered after attn(t)

_CACHE = {}


def _build_program():
    import concourse.bass_interp as _bi

    _orig_sim = _bi.CoreSim.simulate

    def _rec(self, *a, **k):
        r = _orig_sim(self, *a, **k)
        try:
            _CACHE["predicted_ns"] = float(self.time)
        except Exception:
            pass
        return r

    _bi.CoreSim.simulate = _rec
    try:
        return _build_program_inner()
    finally:
        _bi.CoreSim.simulate = _orig_sim


def _build_program_inner():
    nc = bacc.Bacc(
        "TRN2", target_bir_lowering=False, debug=False, num_devices=NCORES
    )

    # ---- I/O ----
    xT = nc.dram_tensor("xT", [D, S], BF16, kind="ExternalInput").ap()
    wqT = nc.dram_tensor("wqT", [128, D], BF16, kind="ExternalInput").ap()
    wkT = nc.dram_tensor("wkT", [128, D], BF16, kind="ExternalInput").ap()
    wvT = nc.dram_tensor("wvT", [128, D], BF16, kind="ExternalInput").ap()
    woT = nc.dram_tensor("woT", [128, D], BF16, kind="ExternalInput").ap()
    cosg = nc.dram_tensor("cosg", [P, S], F32, kind="ExternalInput").ap()
    sing = nc.dram_tensor("sing", [P, S], BF16, kind="ExternalInput").ap()
    masks = nc.dram_tensor("masks", [128, 1408], BF16, kind="ExternalInput").ap()
    ones2 = nc.dram_tensor("ones2", [33, 128], BF16, kind="ExternalInput").ap()
    outT = nc.dram_tensor("outT", [P, S], BF16, kind="ExternalOutput").ap()

    cc_ho_in = [
        nc.dram_tensor(f"cc_ho_in{k}", [P, n * ST], BF16)
        for k, (_, n) in enumerate(RANGES)
    ]
    cc_ho_out = [
        nc.dram_tensor(f"cc_ho_out{k}", [P * NCORES, n * ST], BF16, addr_space="Shared")
        for k, (_, n) in enumerate(RANGES)
    ]
    rg = [list(range(NCORES))]

    with tile.TileContext(nc) as tc:
        with (
            tc.tile_pool(name="const", bufs=1) as constp,
            tc.tile_pool(name="big", bufs=1) as bigp,
            tc.tile_pool(name="xt", bufs=16) as xtld,
            tc.tile_pool(name="trig", bufs=3) as trigp,
            tc.tile_pool(name="rope", bufs=2) as ropep,
            tc.tile_pool(name="pt", bufs=9) as ptp,
            tc.tile_pool(name="dinv", bufs=2) as dinvp,
            tc.tile_pool(name="hb", bufs=1) as hbp,
            tc.tile_pool(name="fout", bufs=3) as foutp,
            tc.tile_pool(name="psum", bufs=2, space="PSUM") as psp,
            tc.tile_pool(name="psum_sc", bufs=2, space="PSUM") as pssc,
            tc.tile_pool(name="psum_o", bufs=2, space="PSUM") as pso,
        ):
            # ---- constants ----
            def load_w(name, src, eng):
                w = constp.tile([128, D], BF16, tag=name)
                eng.dma_start(w[:], src[:])
                return w

            wq_sb = load_w("wq", wqT, nc.sync)
            wk_sb = load_w("wk", wkT, nc.gpsimd)
            wv_sb = load_w("wv", wvT, nc.gpsimd)
            wo_sb = load_w("wo", woT, nc.gpsimd)

            ones2_sb = constp.tile([33, 128], BF16, tag="ones2")
            nc.gpsimd.dma_start(ones2_sb[:], ones2[:])

            mask_sb = constp.tile([128, 1408], BF16, tag="mask")
            nc.gpsimd.dma_start(mask_sb[:], masks[:])

            # ---- big persistent tiles ----
            # q8: [128 grouped-feat, 2S] fp8, columns [0,S)=q, [S,2S)=dup(q)
            # k8: [128 grouped-feat, 2S] fp8, columns [0,S)=k_hi, [S,2S)=k_lo
            q8_sb = bigp.tile([128, 2 * S], F8, tag="q8")
            k8_sb = bigp.tile([128, 2 * S], F8, tag="k8")
            q8v = q8_sb[:].rearrange("p (two s) -> p two s", two=2)
            k8v = k8_sb[:].rearrange("p (two s) -> p two s", two=2)
            # v chunks: per 128-row block B: cols [130B, 130B+130):
            #   h0 v at +0..63, h0 ones at +64, h1 v at +65..128, h1 ones at +129
            v_sb = bigp.tile([128, 130 * (S // 128)], BF16, tag="v")
            nc.vector.memset(v_sb[:], 1.0)
            ho_sb = bigp.tile([128, S], BF16, tag="ho")
            dinv_db = []
            for i in range(2):
                dv = bigp.tile([33, ST], BF16, tag=f"dinv{i}")
                nc.vector.memset(dv[:], 0.0)
                dinv_db.append(dv)

            # ---- projections + rope for s-tile t, as a list of ~1us chunks
            # (pumped between attention pairs to avoid lumpy PE stalls) ----
            def proj_chunks(t):
                xts = []
                for u in range(D // 128):
                    xt_t = xtld.tile([128, ST], BF16)
                    # t=0 runs before attention exists: spread the critical
                    # first loads across idle queues
                    eng = (
                        nc.sync
                        if t > 0
                        else (nc.scalar, nc.sync, nc.gpsimd)[u % 3]
                    )
                    eng.dma_start(
                        xt_t[:],
                        xT[128 * u : 128 * (u + 1), ST * t : ST * (t + 1)],
                    )
                    xts.append(xt_t)

                asl = slice(ST * t, ST * (t + 1))
                dsl = slice(S + ST * t, S + ST * (t + 1))
                teng = nc.scalar if t == 0 else nc.sync
                ct = trigp.tile([P, ST], F32, tag="ct")
                teng.dma_start(ct[:], cosg[:, asl])
                st = trigp.tile([P, ST], BF16, tag="st")
                teng.dma_start(st[:], sing[:, asl])

                def qk_proj(w_sb):
                    pp = psp.tile([128, ST], F32, tag="proj")
                    for u in range(8):
                        nc.tensor.matmul(
                            pp[:],
                            lhsT=w_sb[:, 128 * u : 128 * (u + 1)],
                            rhs=xts[u][:],
                            start=(u == 0),
                            stop=(u == 7),
                        )
                    return pp

                # Pool is free until the first collective: offload early
                # rope/v work there (its TT ops are also cheaper per column)
                veng = nc.gpsimd if t <= 4 else nc.vector

                def rope_common(pp):
                    # swap32(pp) via bf16 copy + 4 partition-swap DMAs
                    pf = ropep.tile([128, ST], BF16, tag="pf")
                    nc.vector.tensor_copy(pf[:], pp[:])
                    psw = ropep.tile([128, ST], BF16, tag="psw")
                    for g in range(4):
                        srow = (g ^ 1) * 32
                        eng = nc.scalar if t == 0 else nc.sync
                        eng.dma_start(
                            psw[32 * g : 32 * (g + 1), :],
                            pf[srow : srow + 32, :],
                        )
                    m1 = ropep.tile([128, ST], F32, tag="m1")
                    nc.vector.tensor_mul(m1[:], pp[:], ct[:])
                    m2 = ropep.tile([128, ST], F32, tag="m2")
                    veng.tensor_mul(m2[:], psw[:], st[:])
                    return m1, m2

                def rope_q(pp):
                    m1, m2 = rope_common(pp)
                    nc.vector.tensor_add(q8_sb[:, asl], m1[:], m2[:])
                    # duplicate for the DoubleRow rhs pair; t=0 is on the
                    # first-score critical path so recompute instead of DMA
                    if t == 0:
                        veng.tensor_add(q8_sb[:, dsl], m1[:], m2[:])
                    else:
                        nc.sync.dma_start(q8_sb[:, dsl], q8_sb[:, asl])

                def rope_k(pp):
                    m1, m2 = rope_common(pp)
                    kf = ropep.tile([128, ST], F32, tag="kf")
                    nc.vector.tensor_add(kf[:], m1[:], m2[:])
                    veng.tensor_copy(k8_sb[:, asl], kf[:])
                    veng.tensor_sub(k8_sb[:, dsl], kf[:], k8_sb[:, asl])

                def v_proj(sx):
                    vp = psp.tile([128, 128], F32, tag="proj")
                    for u in range(8):
                        nc.tensor.matmul(
                            vp[:],
                            lhsT=xts[u][:, 128 * sx : 128 * (sx + 1)],
                            rhs=wv_sb[:, 128 * u : 128 * (u + 1)],
                            start=(u == 0),
                            stop=(u == 7),
                        )
                    B = 4 * t + sx
                    vdst = v_sb[:, 130 * B : 130 * B + 130].rearrange(
                        "p (g c) -> p g c", c=65
                    )[:, :, 0:64]
                    vsrc = vp[:].rearrange("p (g c) -> p g c", c=64)
                    nc.vector.tensor_copy(vdst, vsrc)

                state = {}
                return [
                    lambda: state.__setitem__("q", qk_proj(wq_sb)),
                    lambda: rope_q(state.pop("q")),
                    lambda: state.__setitem__("k", qk_proj(wk_sb)),
                    lambda: rope_k(state.pop("k")),
                    lambda: v_proj(0),
                    lambda: v_proj(1),
                    lambda: v_proj(2),
                    lambda: v_proj(3),
                ]

            def proj_tile(t):
                for c in proj_chunks(t):
                    c()

            # one DoubleRow score matmul: chunk B keys x queries window
            def score_mm(sp, so, h, B, a0, w):
                nc.tensor.matmul(
                    sp[:, so : so + w],
                    lhsT=k8v[64 * h : 64 * h + 64, :, 128 * B : 128 * (B + 1)],
                    rhs=q8v[64 * h : 64 * h + 64, :, a0 : a0 + w],
                    start=True,
                    stop=True,
                    perf_mode=DR,
                )

            # attention for one a-tile, both heads interleaved: while ACT
            # runs one head's exp, PE runs the other head's scores/PV
            def attn_tile2(A, bg=None):
                bg = list(bg or [])
                asl = slice(ST * A, ST * (A + 1))
                nB = 4 * (A + 1)
                op0 = pso.tile([65, ST], F32, tag="o")
                op1 = pso.tile([65, ST], F32, tag="o")
                ops = [op0, op1]

                def pv_pair(h, pB, pt, specs=None):
                    if specs is None:
                        specs = [(ST * i, 0, ST) for i in range(2)]
                    for i in range(2):
                        B = 2 * pB + i
                        so, ao, w = specs[i]
                        nc.tensor.matmul(
                            ops[h][:, ao : ao + w],
                            lhsT=v_sb[:, 130 * B + 65 * h : 130 * B + 65 * h + 65],
                            rhs=pt[:, so : so + w],
                            start=(B == 0),
                            stop=(B == nB - 1),
                        )

                pending = []
                for pB in range(nB // 2):
                    B0 = 2 * pB
                    diag = B0 >= 4 * A
                    dj = B0 - 4 * A
                    if not diag:
                        # full-width pair: (sp_off, a_off_in_tile, width)
                        sspec = [(ST * i, 0, ST) for i in range(2)]
                        pvspec = None
                    elif dj == 0:
                        # chunks 4A, 4A+1: windows a_local [0,512) and [128,512)
                        sspec = [(0, 0, 512), (512, 128, 384)]
                        pvspec = [(0, 0, 512), (512, 128, 384)]
                    else:
                        # chunks 4A+2, 4A+3: both over a_local [256,512)
                        sspec = [(0, 256, 256), (512, 256, 256)]
                        pvspec = [(0, 256, 256), (512, 256, 256)]
                    for h in range(2):
                        sp = pssc.tile([128, 2 * ST], F32, tag="sc")
                        for i in range(2):
                            so, ao, w = sspec[i]
                            score_mm(sp, so, h, B0 + i, ST * A + ao, w)
                        pt = ptp.tile([128, 2 * ST], BF16, tag="pt")
                        if not diag:
                            nc.scalar.activation(pt[:], sp[:], EXPFN, scale=ATT_SCALE)
                        elif dj == 0:
                            pte = ptp.tile([128, 2 * ST], BF16, tag="pte")
                            nc.scalar.activation(pte[:, 0:896], sp[:, 0:896], EXPFN, scale=ATT_SCALE)
                            nc.vector.tensor_mul(
                                pt[:, 0:896], pte[:, 0:896], mask_sb[:, 0:896]
                            )
                        else:
                            # two 256-wide strips at cols 0 and 512 (one bank each)
                            pte = ptp.tile([128, 2 * ST], BF16, tag="pte")
                            spv = sp[:].rearrange("p (g c) -> p g c", c=512)[:, :, 0:256]
                            ptev = pte[:].rearrange("p (g c) -> p g c", c=512)[:, :, 0:256]
                            ptv = pt[:].rearrange("p (g c) -> p g c", c=512)[:, :, 0:256]
                            mkv = mask_sb[:, 896:1408].rearrange(
                                "p (g c) -> p g c", c=256
                            )
                            nc.scalar.activation(ptev, spv, EXPFN, scale=ATT_SCALE)
                            nc.vector.tensor_mul(ptv, ptev, mkv)
                        pvq = 6 if A < NT - 1 or pB < nB // 2 - 2 else 3
                        if len(pending) >= pvq:
                            pv_pair(*pending.pop(0))
                        pending.append((h, pB, pt, pvspec))
                        # small early tiles: pump per head-iteration so the
                        # projection backlog fits inside the attention span
                        if A <= 0 and bg:
                            c2 = bg.pop(0)
                            if c2 is not None:
                                c2()
                    if bg and A > 0:
                        c = bg.pop(0)
                        if c is not None:
                            c()
                while pending:
                    pv_pair(*pending.pop(0))
                # proj chunks must finish before the next tile's attention
                for c in bg:
                    if c is not None:
                        c()
                dinv2 = dinv_db[A % 2]
                with nc.allow_low_precision(reason="denominator bf16 broadcast"):
                    nc.vector.reciprocal(dinv2[0:1, :], ops[0][64:65, :])
                    nc.vector.reciprocal(dinv2[32:33, :], ops[1][64:65, :])
                drep2 = psp.tile([128, ST], F32, tag="proj")
                nc.tensor.matmul(
                    drep2[:], lhsT=ones2_sb[:], rhs=dinv2[:], start=True, stop=True
                )
                dcp = dinvp.tile([128, ST], F32, tag="dcp")
                nc.vector.tensor_copy(dcp[:], drep2[:])
                nc.vector.tensor_mul(ho_sb[0:64, asl], ops[0][0:64, :], dcp[0:64, :])
                nc.vector.tensor_mul(ho_sb[64:128, asl], ops[1][0:64, :], dcp[64:128, :])

            # heads allgather for range k; hb load right after (Pool queue)
            hb_tiles = {}

            def ho_gather(k):
                t0, ntile = RANGES[k]
                w = ntile * ST
                rsl = slice(ST * t0, ST * t0 + w)
                if k == len(RANGES) - 1:
                    hw2 = w // 2
                    nc.sync.dma_start(cc_ho_in[k].ap()[:, 0:hw2], ho_sb[:, ST * t0 : ST * t0 + hw2])
                    nc.scalar.dma_start(
                        cc_ho_in[k].ap()[:, hw2:w], ho_sb[:, ST * t0 + hw2 : ST * t0 + w]
                    )
                else:
                    nc.gpsimd.dma_start(cc_ho_in[k].ap()[:, :], ho_sb[:, rsl])
                nc.gpsimd.collective_compute(
                    "AllGather",
                    mybir.AluOpType.bypass,
                    ins=[cc_ho_in[k].ap()],
                    outs=[cc_ho_out[k].ap()],
                    replica_groups=rg,
                )
                hb = hbp.tile([128, 8 * w], BF16, tag=f"hb{k}")
                hbv = hb[:].rearrange("p (u w) -> p u w", u=8)
                csrc = cc_ho_out[k].ap().rearrange("(u p) w -> p u w", p=128)
                if k == len(RANGES) - 1:
                    engs = (nc.sync, nc.gpsimd, nc.scalar, nc.sync)
                    for uu in range(4):
                        engs[uu].dma_start(
                            hbv[:, 2 * uu : 2 * uu + 2, :],
                            csrc[:, 2 * uu : 2 * uu + 2, :],
                        )
                else:
                    for uu in range(4):
                        nc.gpsimd.dma_start(
                            hbv[:, 2 * uu : 2 * uu + 2, :],
                            csrc[:, 2 * uu : 2 * uu + 2, :],
                        )
                hb_tiles[k] = hbv

            # out-proj for one s-tile of range k (deferred to the tail)
            def outproj_chunk(k, dt_, nhalves=1):
                t0, ntile = RANGES[k]
                t = t0 + dt_
                hbv = hb_tiles[k]
                hw_ = ST // nhalves
                for half in range(nhalves):
                    c0 = ST * dt_ + hw_ * half
                    fp = psp.tile([128, hw_], F32, tag="proj")
                    for u in range(8):
                        nc.tensor.matmul(
                            fp[:],
                            lhsT=wo_sb[:, 128 * u : 128 * (u + 1)],
                            rhs=hbv[:, u, c0 : c0 + hw_],
                            start=(u == 0),
                            stop=(u == 7),
                        )
                    fo = foutp.tile([128, hw_], BF16)
                    nc.vector.tensor_copy(fo[:], fp[:])
                    nc.sync.dma_start(
                        outT[:, ST * t + hw_ * half : ST * t + hw_ * (half + 1)],
                        fo[:],
                    )

            proj_tile(0)
            for t in range(NT):
                bg = []
                if t + 1 < NT:
                    bg += proj_chunks(t + 1)
                attn_tile2(t, bg)
                if t in GATHER_AT:
                    ho_gather(GATHER_AT[t])
            # all output projections here: overlaps the final collective
            for k, (t0, ntile) in enumerate(RANGES):
                if k == len(RANGES) - 1:
                    # keep the PE p-state warm through the final collective's
                    # window so the last out-proj runs at full clock
                    for w_ in range(36):
                        warm = psp.tile([128, ST], F32, tag="proj")
                        nc.tensor.matmul(
                            warm[:],
                            lhsT=wo_sb[:, 0:128],
                            rhs=hb_tiles[0][:, 0, 0:ST],
                            start=True,
                            stop=True,
                        )
                for dt_ in range(ntile):
                    last = k == len(RANGES) - 1 and dt_ == ntile - 1
                    outproj_chunk(k, dt_, nhalves=2 if last else 1)

    nc.compile()
    return nc


def _host_inputs(x, Wq, Wk, Wv, Wo):
    x2 = np.asarray(x).reshape(S, D)
    xTb = np.ascontiguousarray(x2.T).astype(NPBF16)

    # grouped feature permutation per head: pos 64h+32o+f <- orig 64h+2f+o
    perm = np.empty(P, dtype=np.int64)
    for h in range(HPC):
        for o in range(2):
            for f in range(DK // 2):
                perm[DK * h + 32 * o + f] = DK * h + 2 * f + o

    pos = np.arange(S, dtype=np.float64)
    inv_freq = 1.0 / THETA ** (np.arange(0, DK, 2, dtype=np.float64) / DK)
    ang = np.outer(pos, inv_freq)  # [S, 32]
    cos32 = np.cos(ang).T.astype(np.float32)  # [32, S]
    sin32 = np.sin(ang).T.astype(np.float32)
    cosg = np.tile(cos32, (4, 1))  # [128, S] (same for E/O and both heads)
    sing = np.concatenate([-sin32, sin32, -sin32, sin32], axis=0).astype(NPBF16)

    ones2 = np.zeros((33, 128), dtype=NPBF16)
    ones2[0, 0:DK] = 1.0
    ones2[32, DK:128] = 1.0

    bl = np.arange(128)[:, None]
    tri = (bl <= np.arange(128)[None, :]).astype(np.float32)  # [128,128] lower-left
    on = np.ones((128, 128), dtype=np.float32)
    ze = np.zeros((128, 128), dtype=np.float32)
    # p0: j0 [tri|1|1|1] over 512, j1 [tri|1|1] over 384
    # p1: j2 [tri|1] over 256, j3 [0|tri] over 256
    mk = np.concatenate(
        [tri, on, on, on, tri, on, on, tri, on, ze, tri], axis=1
    ).astype(NPBF16)
    assert mk.shape == (128, 1408)

    alpha = 1.03125  # centers a truncating fp8 quantization of q; k exact
    in_maps = []
    for c in range(NCORES):
        rows = slice(P * c, P * (c + 1))
        # 1/sqrt(DK) is folded into the exp activation scale on-device, so q
        # is quantized to fp8 at unit std (fewer subnormals)
        wq_c = (np.asarray(Wq)[rows][perm] * alpha).astype(np.float32)
        wk_c = (np.asarray(Wk)[rows][perm] / alpha).astype(np.float32)
        wv_c = np.asarray(Wv)[rows]
        wo_c = np.asarray(Wo)[rows]  # output rows 128c..128c+128, all input dims

        def pack(wT):
            # wT [1024, 128] -> [128, 1024]: out[p, 128u+j] = wT[128u+p, j]
            return np.ascontiguousarray(
                wT.reshape(8, 128, 128).transpose(1, 0, 2).reshape(128, 1024)
            ).astype(NPBF16)

        in_maps.append(
            {
                "xT": xTb,
                "wqT": pack(wq_c.T),
                "wkT": pack(wk_c.T),
                "wvT": pack(wv_c.T),
                "woT": pack(wo_c.T),
                "cosg": cosg,
                "sing": sing,
                "masks": mk,
                "ones2": ones2,
            }
        )
    return in_maps


def get_program():
    if "nc" not in _CACHE:
        _CACHE["nc"] = _build_program()
    return _CACHE["nc"]


def kernel(x, Wq, Wk, Wv, Wo):
    nc = get_program()
    in_maps = _host_inputs(x, Wq, Wk, Wv, Wo)
    res = run_bass_kernel_spmd(nc, in_maps, list(range(NCORES)))
    out = np.empty((1, S, D), dtype=np.float32)
    for c in range(NCORES):
        out[0, :, P * c : P * (c + 1)] = res.results[c]["outT"].astype(np.float32).T
    return out


if __name__ == "__main__":
    import reference

    inputs = {k: np.asarray(v) for k, v in reference.setup_inputs().items()}
    got = kernel(**inputs)
    exp = np.asarray(reference.reference(**inputs))
    denom = np.abs(exp).max()
    err = np.abs(got - exp).max() / denom
    print(f"Relative error: {err:.3e}")


# revision 52
# speedup vs baseline: 1.0084x; 1.0084x over previous
"""Causal multi-head self-attention (S=4096, D=1024, H=16, RoPE) on 8 trn2 cores.

Tensor-parallel over heads: core c owns heads 2c, 2c+1.
Pipeline per core:
  A) xT shipped pre-transposed from host; per-tile block loads
  B) q/k projections bf16 -> RoPE on DVE -> q stored fp8 (duplicated),
     k stored dual-fp8 (hi + lo residual) for DoubleRow score matmuls
  C) flash-style causal attention: scores via fp8 DoubleRow matmul
     (k_hi.T q + k_lo.T q, half cost), exp on ACT, PV in bf16;
     denominator via ones-column in v; normalize -> headsT
  D) 4 range-wise AllGathers of headsT; ALL output projections deferred
     to overlap the final collective's 15us constant cost
Host assembles out[0, :, 128c:128c+128] = outT_c.T
"""

import sys

for _p in ("/opt/trn_rl_repo", "/root/.axon_site/_ro/trn_rl_repo"):
    if _p not in sys.path:
        sys.path.append(_p)

import numpy as np
import ml_dtypes

import concourse.bass as bass
import concourse.tile as tile
from concourse import bacc, mybir
from concourse.bass_utils import run_bass_kernel_spmd

BF16 = mybir.dt.bfloat16
F32 = mybir.dt.float32
F8 = mybir.dt.float8e4
NPBF16 = ml_dtypes.bfloat16
NPF8 = ml_dtypes.float8_e4m3
DR = mybir.MatmulPerfMode.DoubleRow

S = 4096          # sequence
D = 1024          # model dim
NH = 16           # heads
DK = 64           # head dim
NCORES = 8
HPC = NH // NCORES          # 2 heads per core
P = HPC * DK                # 128 = per-core head feature count
THETA = 10000.0
ST = 512                    # s-tile width (a-tile width too)
NT = S // ST                # 8 tiles
EXPFN = mybir.ActivationFunctionType.Exp
WSCALE = 64.0               # fp8 weight pre-scale (keeps w_lo out of subnormals)
ATT_SCALE = 1.0 / 8.0       # 1/sqrt(DK), folded into the exp activation

# heads-allgather ranges: (first s-tile, n s-tiles). Output projections all
# run after the final gather is issued, filling its 15us constant cost.
RANGES = [(0, 4), (4, 2), (6, 1), (7, 1)]
GATHER_AT = {3: 0, 5: 1, 6: 2, 7: 3}   # t -> range k gathered after attn(t)

_CACHE = {}


def _build_program():
    import concourse.bass_interp as _bi

    _orig_sim = _bi.CoreSim.simulate

    def _rec(self, *a, **k):
        r = _orig_sim(self, *a, **k)
        try:
            _CACHE["predicted_ns"] = float(self.time)
        except Exception:
            pass
        return r

    _bi.CoreSim.simulate = _rec
    try:
        return _build_program_inner()
    finally:
        _bi.CoreSim.simulate = _orig_sim


def _build_program_inner():
    nc = bacc.Bacc(
        "TRN2", target_bir_lowering=False, debug=False, num_devices=NCORES
    )

    # ---- I/O ----
    xT = nc.dram_tensor("xT", [D, S], BF16, kind="ExternalInput").ap()
    wqT = nc.dram_tensor("wqT", [128, D], BF16, kind="ExternalInput").ap()
    wkT = nc.dram_tensor("wkT", [128, D], BF16, kind="ExternalInput").ap()
    wvT = nc.dram_tensor("wvT", [128, D], BF16, kind="ExternalInput").ap()
    woT = nc.dram_tensor("woT", [128, D], BF16, kind="ExternalInput").ap()
    cosg = nc.dram_tensor("cosg", [P, S], F32, kind="ExternalInput").ap()
    sing = nc.dram_tensor("sing", [P, S], BF16, kind="ExternalInput").ap()
    masks = nc.dram_tensor("masks", [128, 1408], BF16, kind="ExternalInput").ap()
    ones2 = nc.dram_tensor("ones2", [33, 128], BF16, kind="ExternalInput").ap()
    outT = nc.dram_tensor("outT", [P, S], BF16, kind="ExternalOutput").ap()

    cc_ho_in = [
        nc.dram_tensor(f"cc_ho_in{k}", [P, n * ST], BF16)
        for k, (_, n) in enumerate(RANGES)
    ]
    cc_ho_out = [
        nc.dram_tensor(f"cc_ho_out{k}", [P * NCORES, n * ST], BF16, addr_space="Shared")
        for k, (_, n) in enumerate(RANGES)
    ]
    rg = [list(range(NCORES))]

    with tile.TileContext(nc) as tc:
        with (
            tc.tile_pool(name="const", bufs=1) as constp,
            tc.tile_pool(name="big", bufs=1) as bigp,
            tc.tile_pool(name="xt", bufs=16) as xtld,
            tc.tile_pool(name="trig", bufs=3) as trigp,
            tc.tile_pool(name="rope", bufs=2) as ropep,
            tc.tile_pool(name="pt", bufs=9) as ptp,
            tc.tile_pool(name="dinv", bufs=2) as dinvp,
            tc.tile_pool(name="hb", bufs=1) as hbp,
            tc.tile_pool(name="fout", bufs=3) as foutp,
            tc.tile_pool(name="psum", bufs=2, space="PSUM") as psp,
            tc.tile_pool(name="psum_sc", bufs=2, space="PSUM") as pssc,
            tc.tile_pool(name="psum_o", bufs=2, space="PSUM") as pso,
        ):
            # ---- constants ----
            def load_w(name, src, eng):
                w = constp.tile([128, D], BF16, tag=name)
                eng.dma_start(w[:], src[:])
                return w

            wq_sb = load_w("wq", wqT, nc.sync)
            wk_sb = load_w("wk", wkT, nc.gpsimd)
            wv_sb = load_w("wv", wvT, nc.gpsimd)

            # ---- big persistent tiles ----
            # q8: [128 grouped-feat, 2S] fp8, columns [0,S)=q, [S,2S)=dup(q)
            # k8: [128 grouped-feat, 2S] fp8, columns [0,S)=k_hi, [S,2S)=k_lo
            q8_sb = bigp.tile([128, 2 * S], F8, tag="q8")
            k8_sb = bigp.tile([128, 2 * S], F8, tag="k8")
            q8v = q8_sb[:].rearrange("p (two s) -> p two s", two=2)
            k8v = k8_sb[:].rearrange("p (two s) -> p two s", two=2)
            # v chunks: per 128-row block B: cols [130B, 130B+130):
            #   h0 v at +0..63, h0 ones at +64, h1 v at +65..128, h1 ones at +129
            v_sb = bigp.tile([128, 130 * (S // 128)], BF16, tag="v")
            nc.vector.memset(v_sb[:], 1.0)
            ho_sb = bigp.tile([128, S], BF16, tag="ho")
            dinv_db = []
            for i in range(2):
                dv = bigp.tile([33, ST], BF16, tag=f"dinv{i}")
                nc.vector.memset(dv[:], 0.0)
                dinv_db.append(dv)

            # ---- projections + rope for s-tile t, as a list of ~1us chunks
            # (pumped between attention pairs to avoid lumpy PE stalls) ----
            def proj_chunks(t):
                xts = []
                for u in range(D // 128):
                    xt_t = xtld.tile([128, ST], BF16)
                    # t=0 runs before attention exists: spread the critical
                    # first loads across idle queues
                    eng = (
                        nc.sync
                        if t > 0
                        else (nc.scalar, nc.sync, nc.gpsimd)[u % 3]
                    )
                    eng.dma_start(
                        xt_t[:],
                        xT[128 * u : 128 * (u + 1), ST * t : ST * (t + 1)],
                    )
                    xts.append(xt_t)

                asl = slice(ST * t, ST * (t + 1))
                dsl = slice(S + ST * t, S + ST * (t + 1))
                teng = nc.scalar if t == 0 else nc.sync
                ct = trigp.tile([P, ST], F32, tag="ct")
                teng.dma_start(ct[:], cosg[:, asl])
                st = trigp.tile([P, ST], BF16, tag="st")
                teng.dma_start(st[:], sing[:, asl])

                def qk_proj(w_sb):
                    pp = psp.tile([128, ST], F32, tag="proj")
                    for u in range(8):
                        nc.tensor.matmul(
                            pp[:],
                            lhsT=w_sb[:, 128 * u : 128 * (u + 1)],
                            rhs=xts[u][:],
                            start=(u == 0),
                            stop=(u == 7),
                        )
                    return pp

                # Pool is free until the first collective: offload early
                # rope/v work there (its TT ops are also cheaper per column)
                veng = nc.gpsimd if t <= 4 else nc.vector

                def rope_common(pp):
                    # swap32(pp) via bf16 copy + 4 partition-swap DMAs
                    pf = ropep.tile([128, ST], BF16, tag="pf")
                    nc.vector.tensor_copy(pf[:], pp[:])
                    psw = ropep.tile([128, ST], BF16, tag="psw")
                    for g in range(4):
                        srow = (g ^ 1) * 32
                        eng = nc.scalar if t == 0 else nc.sync
                        eng.dma_start(
                            psw[32 * g : 32 * (g + 1), :],
                            pf[srow : srow + 32, :],
                        )
                    m1 = ropep.tile([128, ST], F32, tag="m1")
                    nc.vector.tensor_mul(m1[:], pp[:], ct[:])
                    m2 = ropep.tile([128, ST], F32, tag="m2")
                    veng.tensor_mul(m2[:], psw[:], st[:])
                    return m1, m2

                def rope_q(pp):
                    m1, m2 = rope_common(pp)
                    nc.vector.tensor_add(q8_sb[:, asl], m1[:], m2[:])
                    # duplicate for the DoubleRow rhs pair; t=0 is on the
                    # first-score critical path so recompute instead of DMA
                    if t == 0:
                        veng.tensor_add(q8_sb[:, dsl], m1[:], m2[:])
                    else:
                        nc.sync.dma_start(q8_sb[:, dsl], q8_sb[:, asl])

                def rope_k(pp):
                    m1, m2 = rope_common(pp)
                    kf = ropep.tile([128, ST], F32, tag="kf")
                    nc.vector.tensor_add(kf[:], m1[:], m2[:])
                    veng.tensor_copy(k8_sb[:, asl], kf[:])
                    veng.tensor_sub(k8_sb[:, dsl], kf[:], k8_sb[:, asl])

                def v_proj(sx):
                    vp = psp.tile([128, 128], F32, tag="proj")
                    for u in range(8):
                        nc.tensor.matmul(
                            vp[:],
                            lhsT=xts[u][:, 128 * sx : 128 * (sx + 1)],
                            rhs=wv_sb[:, 128 * u : 128 * (u + 1)],
                            start=(u == 0),
                            stop=(u == 7),
                        )
                    B = 4 * t + sx
                    vdst = v_sb[:, 130 * B : 130 * B + 130].rearrange(
                        "p (g c) -> p g c", c=65
                    )[:, :, 0:64]
                    vsrc = vp[:].rearrange("p (g c) -> p g c", c=64)
                    nc.vector.tensor_copy(vdst, vsrc)

                state = {}
                return [
                    lambda: state.__setitem__("q", qk_proj(wq_sb)),
                    lambda: rope_q(state.pop("q")),
                    lambda: state.__setitem__("k", qk_proj(wk_sb)),
                    lambda: rope_k(state.pop("k")),
                    lambda: v_proj(0),
                    lambda: v_proj(1),
                    lambda: v_proj(2),
                    lambda: v_proj(3),
                ]

            def proj_tile(t):
                for c in proj_chunks(t):
                    c()

            # one DoubleRow score matmul: chunk B keys x queries window
            def score_mm(sp, so, h, B, a0, w):
                nc.tensor.matmul(
                    sp[:, so : so + w],
                    lhsT=k8v[64 * h : 64 * h + 64, :, 128 * B : 128 * (B + 1)],
                    rhs=q8v[64 * h : 64 * h + 64, :, a0 : a0 + w],
                    start=True,
                    stop=True,
                    perf_mode=DR,
                )

            # attention for one a-tile, both heads interleaved: while ACT
            # runs one head's exp, PE runs the other head's scores/PV
            def attn_tile2(A, bg=None):
                bg = list(bg or [])
                asl = slice(ST * A, ST * (A + 1))
                nB = 4 * (A + 1)
                op0 = pso.tile([65, ST], F32, tag="o")
                op1 = pso.tile([65, ST], F32, tag="o")
                ops = [op0, op1]

                def pv_pair(h, pB, pt, specs=None):
                    if specs is None:
                        specs = [(ST * i, 0, ST) for i in range(2)]
                    for i in range(2):
                        B = 2 * pB + i
                        so, ao, w = specs[i]
                        nc.tensor.matmul(
                            ops[h][:, ao : ao + w],
                            lhsT=v_sb[:, 130 * B + 65 * h : 130 * B + 65 * h + 65],
                            rhs=pt[:, so : so + w],
                            start=(B == 0),
                            stop=(B == nB - 1),
                        )

                pending = []
                for pB in range(nB // 2):
                    B0 = 2 * pB
                    diag = B0 >= 4 * A
                    dj = B0 - 4 * A
                    if not diag:
                        # full-width pair: (sp_off, a_off_in_tile, width)
                        sspec = [(ST * i, 0, ST) for i in range(2)]
                        pvspec = None
                    elif dj == 0:
                        # chunks 4A, 4A+1: windows a_local [0,512) and [128,512)
                        sspec = [(0, 0, 512), (512, 128, 384)]
                        pvspec = [(0, 0, 512), (512, 128, 384)]
                    else:
                        # chunks 4A+2, 4A+3: both over a_local [256,512)
                        sspec = [(0, 256, 256), (512, 256, 256)]
                        pvspec = [(0, 256, 256), (512, 256, 256)]
                    for h in range(2):
                        sp = pssc.tile([128, 2 * ST], F32, tag="sc")
                        for i in range(2):
                            so, ao, w = sspec[i]
                            score_mm(sp, so, h, B0 + i, ST * A + ao, w)
                        pt = ptp.tile([128, 2 * ST], BF16, tag="pt")
                        if not diag:
                            nc.scalar.activation(pt[:], sp[:], EXPFN, scale=ATT_SCALE)
                        elif dj == 0:
                            pte = ptp.tile([128, 2 * ST], BF16, tag="pte")
                            nc.scalar.activation(pte[:, 0:896], sp[:, 0:896], EXPFN, scale=ATT_SCALE)
                            nc.vector.tensor_mul(
                                pt[:, 0:896], pte[:, 0:896], mask_sb[:, 0:896]
                            )
                        else:
                            # two 256-wide strips at cols 0 and 512 (one bank each)
                            pte = ptp.tile([128, 2 * ST], BF16, tag="pte")
                            spv = sp[:].rearrange("p (g c) -> p g c", c=512)[:, :, 0:256]
                            ptev = pte[:].rearrange("p (g c) -> p g c", c=512)[:, :, 0:256]
                            ptv = pt[:].rearrange("p (g c) -> p g c", c=512)[:, :, 0:256]
                            mkv = mask_sb[:, 896:1408].rearrange(
                                "p (g c) -> p g c", c=256
                            )
                            nc.scalar.activation(ptev, spv, EXPFN, scale=ATT_SCALE)
                            nc.vector.tensor_mul(ptv, ptev, mkv)
                        pvq = 6 if A < NT - 1 or pB < nB // 2 - 2 else 3
                        if len(pending) >= pvq:
                            pv_pair(*pending.pop(0))
                        pending.append((h, pB, pt, pvspec))
                        # small early tiles: pump per head-iteration so the
                        # projection backlog fits inside the attention span
                        if A <= 0 and bg:
                            c2 = bg.pop(0)
                            if c2 is not None:
                                c2()
                    if bg and A > 0:
                        c = bg.pop(0)
                        if c is not None:
                            c()
                while pending:
                    pv_pair(*pending.pop(0))
                # proj chunks must finish before the next tile's attention
                for c in bg:
                    if c is not None:
                        c()
                dinv2 = dinv_db[A % 2]
                with nc.allow_low_precision(reason="denominator bf16 broadcast"):
                    nc.vector.reciprocal(dinv2[0:1, :], ops[0][64:65, :])
                    nc.vector.reciprocal(dinv2[32:33, :], ops[1][64:65, :])
                drep2 = psp.tile([128, ST], F32, tag="proj")
                nc.tensor.matmul(
                    drep2[:], lhsT=ones2_sb[:], rhs=dinv2[:], start=True, stop=True
                )
                dcp = dinvp.tile([128, ST], F32, tag="dcp")
                nc.vector.tensor_copy(dcp[:], drep2[:])
                nc.vector.tensor_mul(ho_sb[0:64, asl], ops[0][0:64, :], dcp[0:64, :])
                nc.vector.tensor_mul(ho_sb[64:128, asl], ops[1][0:64, :], dcp[64:128, :])

            # heads allgather for range k; hb load right after (Pool queue)
            hb_tiles = {}

            def ho_gather(k):
                t0, ntile = RANGES[k]
                w = ntile * ST
                rsl = slice(ST * t0, ST * t0 + w)
                if k == len(RANGES) - 1:
                    hw2 = w // 2
                    nc.sync.dma_start(cc_ho_in[k].ap()[:, 0:hw2], ho_sb[:, ST * t0 : ST * t0 + hw2])
                    nc.scalar.dma_start(
                        cc_ho_in[k].ap()[:, hw2:w], ho_sb[:, ST * t0 + hw2 : ST * t0 + w]
                    )
                else:
                    nc.gpsimd.dma_start(cc_ho_in[k].ap()[:, :], ho_sb[:, rsl])
                nc.gpsimd.collective_compute(
                    "AllGather",
                    mybir.AluOpType.bypass,
                    ins=[cc_ho_in[k].ap()],
                    outs=[cc_ho_out[k].ap()],
                    replica_groups=rg,
                )
                hb = hbp.tile([128, 8 * w], BF16, tag=f"hb{k}")
                hbv = hb[:].rearrange("p (u w) -> p u w", u=8)
                csrc = cc_ho_out[k].ap().rearrange("(u p) w -> p u w", p=128)
                if k == len(RANGES) - 1:
                    engs = (nc.sync, nc.gpsimd, nc.scalar, nc.sync)
                    for uu in range(4):
                        engs[uu].dma_start(
                            hbv[:, 2 * uu : 2 * uu + 2, :],
                            csrc[:, 2 * uu : 2 * uu + 2, :],
                        )
                else:
                    for uu in range(4):
                        nc.gpsimd.dma_start(
                            hbv[:, 2 * uu : 2 * uu + 2, :],
                            csrc[:, 2 * uu : 2 * uu + 2, :],
                        )
                hb_tiles[k] = hbv

            # out-proj for one s-tile of range k (deferred to the tail)
            def outproj_chunk(k, dt_, nhalves=1):
                t0, ntile = RANGES[k]
                t = t0 + dt_
                hbv = hb_tiles[k]
                hw_ = ST // nhalves
                for half in range(nhalves):
                    c0 = ST * dt_ + hw_ * half
                    fp = psp.tile([128, hw_], F32, tag="proj")
                    for u in range(8):
                        nc.tensor.matmul(
                            fp[:],
                            lhsT=wo_sb[:, 128 * u : 128 * (u + 1)],
                            rhs=hbv[:, u, c0 : c0 + hw_],
                            start=(u == 0),
                            stop=(u == 7),
                        )
                    fo = foutp.tile([128, hw_], BF16)
                    nc.vector.tensor_copy(fo[:], fp[:])
                    nc.sync.dma_start(
                        outT[:, ST * t + hw_ * half : ST * t + hw_ * (half + 1)],
                        fo[:],
                    )

            proj_tile(0)
            # non-startup-critical constants: emitted after tile 0's chain so
            # they queue behind it, not ahead of it
            wo_sb = load_w("wo", woT, nc.gpsimd)
            ones2_sb = constp.tile([33, 128], BF16, tag="ones2")
            nc.gpsimd.dma_start(ones2_sb[:], ones2[:])
            mask_sb = constp.tile([128, 1408], BF16, tag="mask")
            nc.gpsimd.dma_start(mask_sb[:], masks[:])
            for t in range(NT):
                bg = []
                if t + 1 < NT:
                    bg += proj_chunks(t + 1)
                attn_tile2(t, bg)
                if t in GATHER_AT:
                    ho_gather(GATHER_AT[t])
            # all output projections here: overlaps the final collective
            for k, (t0, ntile) in enumerate(RANGES):
                if k == len(RANGES) - 1:
                    # keep the PE p-state warm through the final collective's
                    # window so the last out-proj runs at full clock
                    for w_ in range(36):
                        warm = psp.tile([128, ST], F32, tag="proj")
                        nc.tensor.matmul(
                            warm[:],
                            lhsT=wo_sb[:, 0:128],
                            rhs=hb_tiles[0][:, 0, 0:ST],
                            start=True,
                            stop=True,
                        )
                for dt_ in range(ntile):
                    last = k == len(RANGES) - 1 and dt_ == ntile - 1
                    outproj_chunk(k, dt_, nhalves=2 if last else 1)

    nc.compile()
    return nc


def _host_inputs(x, Wq, Wk, Wv, Wo):
    x2 = np.asarray(x).reshape(S, D)
    xTb = np.ascontiguousarray(x2.T).astype(NPBF16)

    # grouped feature permutation per head: pos 64h+32o+f <- orig 64h+2f+o
    perm = np.empty(P, dtype=np.int64)
    for h in range(HPC):
        for o in range(2):
            for f in range(DK // 2):
                perm[DK * h + 32 * o + f] = DK * h + 2 * f + o

    pos = np.arange(S, dtype=np.float64)
    inv_freq = 1.0 / THETA ** (np.arange(0, DK, 2, dtype=np.float64) / DK)
    ang = np.outer(pos, inv_freq)  # [S, 32]
    cos32 = np.cos(ang).T.astype(np.float32)  # [32, S]
    sin32 = np.sin(ang).T.astype(np.float32)
    cosg = np.tile(cos32, (4, 1))  # [128, S] (same for E/O and both heads)
    sing = np.concatenate([-sin32, sin32, -sin32, sin32], axis=0).astype(NPBF16)

    ones2 = np.zeros((33, 128), dtype=NPBF16)
    ones2[0, 0:DK] = 1.0
    ones2[32, DK:128] = 1.0

    bl = np.arange(128)[:, None]
    tri = (bl <= np.arange(128)[None, :]).astype(np.float32)  # [128,128] lower-left
    on = np.ones((128, 128), dtype=np.float32)
    ze = np.zeros((128, 128), dtype=np.float32)
    # p0: j0 [tri|1|1|1] over 512, j1 [tri|1|1] over 384
    # p1: j2 [tri|1] over 256, j3 [0|tri] over 256
    mk = np.concatenate(
        [tri, on, on, on, tri, on, on, tri, on, ze, tri], axis=1
    ).astype(NPBF16)
    assert mk.shape == (128, 1408)

    alpha = 1.03125  # centers a truncating fp8 quantization of q; k exact
    in_maps = []
    for c in range(NCORES):
        rows = slice(P * c, P * (c + 1))
        # 1/sqrt(DK) is folded into the exp activation scale on-device, so q
        # is quantized to fp8 at unit std (fewer subnormals)
        wq_c = (np.asarray(Wq)[rows][perm] * alpha).astype(np.float32)
        wk_c = (np.asarray(Wk)[rows][perm] / alpha).astype(np.float32)
        wv_c = np.asarray(Wv)[rows]
        wo_c = np.asarray(Wo)[rows]  # output rows 128c..128c+128, all input dims

        def pack(wT):
            # wT [1024, 128] -> [128, 1024]: out[p, 128u+j] = wT[128u+p, j]
            return np.ascontiguousarray(
                wT.reshape(8, 128, 128).transpose(1, 0, 2).reshape(128, 1024)
            ).astype(NPBF16)

        in_maps.append(
            {
                "xT": xTb,
                "wqT": pack(wq_c.T),
                "wkT": pack(wk_c.T),
                "wvT": pack(wv_c.T),
                "woT": pack(wo_c.T),
                "cosg": cosg,
                "sing": sing,
                "masks": mk,
                "ones2": ones2,
            }
        )
    return in_maps


def get_program():
    if "nc" not in _CACHE:
        _CACHE["nc"] = _build_program()
    return _CACHE["nc"]


def kernel(x, Wq, Wk, Wv, Wo):
    nc = get_program()
    in_maps = _host_inputs(x, Wq, Wk, Wv, Wo)
    res = run_bass_kernel_spmd(nc, in_maps, list(range(NCORES)))
    out = np.empty((1, S, D), dtype=np.float32)
    for c in range(NCORES):
        out[0, :, P * c : P * (c + 1)] = res.results[c]["outT"].astype(np.float32).T
    return out


if __name__ == "__main__":
    import reference

    inputs = {k: np.asarray(v) for k, v in reference.setup_inputs().items()}
    got = kernel(**inputs)
    exp = np.asarray(reference.reference(**inputs))
    denom = np.abs(exp).max()
    err = np.abs(got - exp).max() / denom
    print(f"Relative error: {err:.3e}")
